# revision 21
# baseline (speedup 1.0000x reference)
"""ArcNegFace loss kernel for 8 TRN2 NeuronCores.

Model-parallel classification head: weight [100000, 512] is sharded over
out_features across 8 cores (padded to 102400 -> 12800 cols/core).

Host-side prep (sharding/layout, like the label gather):
  - L2-normalize weight rows in f32, scale by 16 and quantize to
    fp8 e4m3, laid out chunk-major in the DoubleRow [Ki, Ko=2, dim]
    interleave: wt[p, kp, ko, c] = 16*wn[c, kp*256 + ko*128 + p]
  - normalize input rows, scale by 16, quantize e4m3:
    xt[p, kp, ko, j2, b] = 16*xn[j2*128+b, kp*256+ko*128+p]
  - a_lb (256 margined target logits) computed on host from the f32
    normalized rows; the one-hot positive logits are patched during
    unsharding.

Device per core (fully streaming):
  HBM --HWDGE--> wt chunk [128, 2, 2, cols] fp8e4
  pc   = 256 * xnT.T @ wt      (PE, K=512 as 2 PSUM-accum DoubleRow
                                matmuls, K=256 each: 2 fp8 values per
                                PE cell, 2x throughput)
  f'   = Derivative_Erf((pc/256 - a)/sqrt(2))     (ACT, PSUM src,
         = 2/sqrt(pi) * exp(-(cos-a)^2/sigma)      per-partition bias)
  s'   = (pc + 256) * f'       (DVE scalar_tensor_tensor, fp16
                                = 256*(cos+1)*f')
  HBM <-- s' [128, 2, cols] fp16   (one store per chunk, SWDGE queue)

The affine tail  logits = (K2/256)*s' - SCALE  (K2 = SCALE*ALPHA*
sqrt(pi)/2) is folded into the host's fp16->f32 unshard pass.
"""

import math

import numpy as np

try:
    from ml_dtypes import float8_e4m3 as _f8e4
except ImportError:  # pragma: no cover
    _f8e4 = None

B, D, C = 256, 512, 100000
NCORES = 8
CSH = 12800                 # padded columns per core
CPAD = CSH * NCORES        # 102400
CHUNKS = [512, 2048, 2048, 2048, 2048, 2048, 1024, 512, 512]  # sum = 12800
WS = 16.0                  # fp8 weight pre-scale (power of 2)
XS = 16.0                  # fp8 input pre-scale (power of 2)
SCALE = 64.0
MARGIN = 0.5
ALPHA = 1.2
SIGMA = 2.0
THRESH = math.cos(math.pi - MARGIN)
MM_ = math.sin(math.pi - MARGIN) * MARGIN
K1 = SCALE * ALPHA
K2 = K1 * math.sqrt(math.pi) / 2.0
RSQ2 = 1.0 / math.sqrt(2.0)

_CACHE: dict = {}


def _build():
    from contextlib import ExitStack

    import concourse.bacc as bacc
    import concourse.tile as tile
    from concourse import mybir

    f32 = mybir.dt.float32
    f16 = mybir.dt.float16
    f8e4 = mybir.dt.float8e4
    Alu = mybir.AluOpType
    Act = mybir.ActivationFunctionType
    DR = mybir.MatmulPerfMode.DoubleRow

    nc = bacc.Bacc(
        "TRN2", target_bir_lowering=False, debug=False, num_devices=NCORES
    )
    xt_e = nc.dram_tensor("xt", [128, 2, 2, 2, 128], f8e4,
                          kind="ExternalInput").ap()
    na_e = nc.dram_tensor("na", [128, 2], f32, kind="ExternalInput").ap()
    wt_e = nc.dram_tensor("wt", [128, 4 * CSH], f8e4,
                          kind="ExternalInput").ap()
    out_e = nc.dram_tensor("out", [B, CSH], f16, kind="ExternalOutput").ap()
    out_r = out_e.rearrange("(j p) c -> p j c", p=128)

    with tile.TileContext(nc) as tc, ExitStack() as ctx:
        singles = ctx.enter_context(tc.tile_pool(name="singles", bufs=1))
        wpool = ctx.enter_context(tc.tile_pool(name="wpool", bufs=9))
        epool = ctx.enter_context(tc.tile_pool(name="epool", bufs=4))
        opool = ctx.enter_context(tc.tile_pool(name="opool", bufs=4))
        psum = ctx.enter_context(tc.tile_pool(name="psum", bufs=4, space="PSUM"))

        # first weight chunk DMA goes out first; xt/na are tiny and follow
        wt0 = wpool.tile([128, 2, 2, CHUNKS[0]], f8e4, tag="wt", name="wt0")
        nc.sync.dma_start(
            wt0, wt_e[:, :4 * CHUNKS[0]].rearrange(
                "p (a b c) -> p a b c", a=2, b=2))
        xt = singles.tile([128, 2, 2, 2, 128], f8e4)
        nc.sync.dma_start(xt, xt_e)
        na = singles.tile([128, 2], f32)
        nc.sync.dma_start(na, na_e)

        c0 = 0
        stt_i = 0
        for ci, cols in enumerate(CHUNKS):
            if ci == 0:
                wt = wt0
            else:
                wt = wpool.tile([128, 2, 2, cols], f8e4, tag="wt",
                                name=f"wt{ci}")
                nc.sync.dma_start(
                    wt, wt_e[:, 4 * c0:4 * (c0 + cols)].rearrange(
                        "p (a b c) -> p a b c", a=2, b=2))
            ot = opool.tile([128, 2, cols], f16, tag="ot", name=f"ot{ci}")
            # subtile layout: 1024-col psum tiles (+ a 512 remainder)
            subs = []
            s0 = 0
            while s0 < cols:
                nsz = 1024 if cols - s0 >= 1024 else cols - s0
                subs.append((s0, nsz))
                s0 += nsz
            for j2 in range(2):
                pcs = [psum.tile([128, nsz], f32, tag="pc",
                                 name=f"pc{ci}_{si}_{j2}")
                       for si, (s0, nsz) in enumerate(subs)]
                # kp-outer: each stationary x block feeds every rhs slice
                # of the chunk (accumulation groups interleave across
                # psum tiles, hence skip_group_check).
                for kp in range(2):
                    for si, (s0, nsz) in enumerate(subs):
                        for h in range((nsz + 511) // 512):
                            hw = min(512, nsz - h * 512)
                            nc.tensor.matmul(
                                pcs[si][:, h * 512:h * 512 + hw],
                                lhsT=xt[:, kp, :, j2, :],
                                rhs=wt[:, kp, :,
                                       s0 + h * 512:s0 + h * 512 + hw],
                                start=(kp == 0), stop=(kp == 1),
                                perf_mode=DR,
                                skip_group_check=True)
                for si, (s0, nsz) in enumerate(subs):
                    f_ = epool.tile([128, nsz], f16, tag="f",
                                    name=f"f_{ci}_{si}_{j2}")
                    nc.scalar.activation(f_, pcs[si], Act.Derivative_Erf,
                                         bias=na[:, j2:j2 + 1],
                                         scale=RSQ2 / (WS * XS))
                    nc.vector.scalar_tensor_tensor(
                        ot[:, j2, s0:s0 + nsz], pcs[si], WS * XS, f_,
                        Alu.add, Alu.mult)
            # last two stores go HWDGE so the SWDGE ring drain (a ~4us
            # GpSimd DRAIN) starts early and overlaps the remaining work
            # instead of blocking the end-of-kernel barrier.
            if ci >= len(CHUNKS) - 3:
                nc.sync.dma_start(out_r[:, :, c0:c0 + cols], ot)
            else:
                nc.gpsimd.dma_start(out_r[:, :, c0:c0 + cols], ot)
            c0 += cols

    nc.compile()
    return nc


def _get_nc():
    nc = _CACHE.get("nc")
    if nc is None:
        nc = _build()
        _CACHE["nc"] = nc
    return nc


def _run(in_maps, trace=False, tmpdir=None):
    from concourse.bass_utils import run_bass_kernel_spmd

    nc = _get_nc()
    return run_bass_kernel_spmd(
        nc, in_maps, core_ids=list(range(NCORES)), trace=trace, tmpdir=tmpdir)


def make_in_maps(input, label, weight):
    inp = np.asarray(input, dtype=np.float32)
    lab = np.asarray(label).astype(np.int64)
    w = np.asarray(weight, dtype=np.float32)

    wpad = np.concatenate([w, np.ones((CPAD - C, D), np.float32)], axis=0)
    rnorm = 1.0 / np.maximum(np.linalg.norm(wpad, axis=1), 1e-12)
    wn = wpad * rnorm[:, None]

    xnorm = 1.0 / np.maximum(np.linalg.norm(inp, axis=1), 1e-12)
    xn = inp * xnorm[:, None]

    # a_lb from exact f32 normalized rows
    cos_lb = np.einsum("bd,bd->b", xn, wn[lab], dtype=np.float64)
    a_lb = np.where(
        cos_lb > THRESH,
        np.cos(np.arccos(np.clip(cos_lb, -1.0, 1.0)) + MARGIN),
        cos_lb - MM_,
    ).astype(np.float32)
    # ACT bias: -a/sqrt(2), per partition; row r = j*128 + p
    na = np.ascontiguousarray((-a_lb * RSQ2).reshape(2, 128).T)   # [128, 2]

    # xt[p, kp, ko, j2, b] = XS * xn[j2*128 + b, kp*256 + ko*128 + p]
    xt = np.ascontiguousarray(
        (xn * XS).astype(_f8e4).T.reshape(2, 2, 128, 2, 128)
        .transpose(2, 0, 1, 3, 4))

    # wt chunk-major DoubleRow layout: per chunk block [128, 2, 2, cols]
    # with wt[p, kp, ko, c] = WS * wn[c0 + c, kp*256 + ko*128 + p]
    wt_full = ((wn * WS).astype(_f8e4).T
               .reshape(2, 2, 128, CPAD).transpose(2, 0, 1, 3))
    in_maps = []
    for i in range(NCORES):
        sl = wt_full[:, :, :, i * CSH:(i + 1) * CSH]
        blocks = []
        c0 = 0
        for cols in CHUNKS:
            blocks.append(sl[:, :, :, c0:c0 + cols].reshape(128, 4 * cols))
            c0 += cols
        in_maps.append(
            {"xt": xt, "na": na,
             "wt": np.ascontiguousarray(np.concatenate(blocks, axis=1))})
    return in_maps, (lab, a_lb)


def assemble(results, aux):
    lab, a_lb = aux
    s = np.concatenate(
        [results[i]["out"] for i in range(NCORES)], axis=1
    )[:, :C]
    full = s.astype(np.float32) * np.float32(K2 / (WS * XS)) - np.float32(SCALE)
    full[np.arange(B), lab] = (SCALE * a_lb).astype(np.float32)
    return full


def kernel(input, label, weight):
    in_maps, aux = make_in_maps(input, label, weight)
    res = _run(in_maps)
    return assemble(res.results, aux)


# revision 22
# speedup vs baseline: 1.1167x; 1.1167x over previous
"""ArcNegFace loss kernel for 8 TRN2 NeuronCores.

Model-parallel classification head: weight [100000, 512] is sharded over
out_features across 8 cores (padded to 102400 -> 12800 cols/core).

Host-side prep (sharding/layout, like the label gather):
  - L2-normalize weight rows in f32, scale by 16 and quantize to
    fp8 e4m3, laid out chunk-major in the DoubleRow [Ki, Ko=2, dim]
    interleave: wt[p, kp, ko, c] = 16*wn[c, kp*256 + ko*128 + p]
  - normalize input rows, scale by 16, quantize e4m3:
    xt[p, kp, ko, j2, b] = 16*xn[j2*128+b, kp*256+ko*128+p]
  - a_lb (256 margined target logits) computed on host from the f32
    normalized rows; the one-hot positive logits are patched during
    unsharding.

Device per core (fully streaming):
  HBM --HWDGE--> wt chunk [128, 2, 2, cols] fp8e4
  pc   = 256 * xnT.T @ wt      (PE, K=512 as 2 PSUM-accum DoubleRow
                                matmuls, K=256 each: 2 fp8 values per
                                PE cell, 2x throughput)
  f'   = Derivative_Erf((pc/256 - a)/sqrt(2))     (ACT, PSUM src,
         = 2/sqrt(pi) * exp(-(cos-a)^2/sigma)      per-partition bias)
  s'   = (pc + 256) * f'       (DVE scalar_tensor_tensor, fp16
                                = 256*(cos+1)*f')
  HBM <-- s' [128, 2, cols] fp16   (one store per chunk, SWDGE queue)

The affine tail  logits = (K2/256)*s' - SCALE  (K2 = SCALE*ALPHA*
sqrt(pi)/2) is folded into the host's fp16->f32 unshard pass.
"""

import math

import numpy as np

try:
    from ml_dtypes import float8_e4m3 as _f8e4
except ImportError:  # pragma: no cover
    _f8e4 = None

B, D, C = 256, 512, 100000
NCORES = 8
CSH = 12800                 # padded columns per core
CPAD = CSH * NCORES        # 102400
CHUNKS = [512, 2048, 2048, 2048, 2048, 2048, 1024, 512, 512]  # sum = 12800
WS = 16.0                  # fp8 weight pre-scale (power of 2)
XS = 16.0                  # fp8 input pre-scale (power of 2)
SCALE = 64.0
MARGIN = 0.5
ALPHA = 1.2
SIGMA = 2.0
THRESH = math.cos(math.pi - MARGIN)
MM_ = math.sin(math.pi - MARGIN) * MARGIN
K1 = SCALE * ALPHA
K2 = K1 * math.sqrt(math.pi) / 2.0
RSQ2 = 1.0 / math.sqrt(2.0)

_CACHE: dict = {}


def _build():
    from contextlib import ExitStack

    import concourse.bacc as bacc
    import concourse.tile as tile
    from concourse import mybir

    f32 = mybir.dt.float32
    f16 = mybir.dt.float16
    f8e4 = mybir.dt.float8e4
    Alu = mybir.AluOpType
    Act = mybir.ActivationFunctionType
    DR = mybir.MatmulPerfMode.DoubleRow

    nc = bacc.Bacc(
        "TRN2", target_bir_lowering=False, debug=False, num_devices=NCORES
    )
    xt_e = nc.dram_tensor("xt", [128, 2, 2, 2, 128], f8e4,
                          kind="ExternalInput").ap()
    na_e = nc.dram_tensor("na", [128, 2], f32, kind="ExternalInput").ap()
    wt_e = nc.dram_tensor("wt", [128, 4 * CSH], f8e4,
                          kind="ExternalInput").ap()
    out_e = nc.dram_tensor("out", [B, CSH], f16, kind="ExternalOutput").ap()
    out_r = out_e.rearrange("(j p) c -> p j c", p=128)

    with tile.TileContext(nc) as tc, ExitStack() as ctx:
        singles = ctx.enter_context(tc.tile_pool(name="singles", bufs=1))
        wpool = ctx.enter_context(tc.tile_pool(name="wpool", bufs=5))
        epool = ctx.enter_context(tc.tile_pool(name="epool", bufs=4))
        opool = ctx.enter_context(tc.tile_pool(name="opool", bufs=4))
        psum = ctx.enter_context(tc.tile_pool(name="psum", bufs=4, space="PSUM"))

        # first weight chunk DMA goes out first; xt/na are tiny and follow
        wt0 = wpool.tile([128, 2, 2, CHUNKS[0]], f8e4, tag="wt", name="wt0")
        nc.sync.dma_start(
            wt0, wt_e[:, :4 * CHUNKS[0]].rearrange(
                "p (a b c) -> p a b c", a=2, b=2))
        xt = singles.tile([128, 2, 2, 2, 128], f8e4)
        nc.sync.dma_start(xt, xt_e)
        na = singles.tile([128, 2], f32)
        nc.sync.dma_start(na, na_e)

        c0 = 0
        stt_i = 0
        for ci, cols in enumerate(CHUNKS):
            if ci == 0:
                wt = wt0
            else:
                wt = wpool.tile([128, 2, 2, cols], f8e4, tag="wt",
                                name=f"wt{ci}")
                nc.sync.dma_start(
                    wt, wt_e[:, 4 * c0:4 * (c0 + cols)].rearrange(
                        "p (a b c) -> p a b c", a=2, b=2))
            ot = opool.tile([128, 2, cols], f16, tag="ot", name=f"ot{ci}")
            # subtile layout: 1024-col psum tiles (+ a 512 remainder)
            subs = []
            s0 = 0
            while s0 < cols:
                nsz = 1024 if cols - s0 >= 1024 else cols - s0
                subs.append((s0, nsz))
                s0 += nsz
            for j2 in range(2):
                pcs = [psum.tile([128, nsz], f32, tag="pc",
                                 name=f"pc{ci}_{si}_{j2}")
                       for si, (s0, nsz) in enumerate(subs)]
                # kp-outer: each stationary x block feeds every rhs slice
                # of the chunk (accumulation groups interleave across
                # psum tiles, hence skip_group_check).
                for kp in range(2):
                    for si, (s0, nsz) in enumerate(subs):
                        for h in range((nsz + 511) // 512):
                            hw = min(512, nsz - h * 512)
                            nc.tensor.matmul(
                                pcs[si][:, h * 512:h * 512 + hw],
                                lhsT=xt[:, kp, :, j2, :],
                                rhs=wt[:, kp, :,
                                       s0 + h * 512:s0 + h * 512 + hw],
                                start=(kp == 0), stop=(kp == 1),
                                perf_mode=DR,
                                skip_group_check=True)
                for si, (s0, nsz) in enumerate(subs):
                    f_ = epool.tile([128, nsz], f16, tag="f",
                                    name=f"f_{ci}_{si}_{j2}")
                    nc.scalar.activation(f_, pcs[si], Act.Derivative_Erf,
                                         bias=na[:, j2:j2 + 1],
                                         scale=RSQ2 / (WS * XS))
                    nc.vector.scalar_tensor_tensor(
                        ot[:, j2, s0:s0 + nsz], pcs[si], WS * XS, f_,
                        Alu.add, Alu.mult)
            # last two stores go HWDGE so the SWDGE ring drain (a ~4us
            # GpSimd DRAIN) starts early and overlaps the remaining work
            # instead of blocking the end-of-kernel barrier.
            if ci >= len(CHUNKS) - 3:
                nc.sync.dma_start(out_r[:, :, c0:c0 + cols], ot)
            else:
                nc.gpsimd.dma_start(out_r[:, :, c0:c0 + cols], ot)
            c0 += cols

    nc.compile()
    return nc


def _get_nc():
    nc = _CACHE.get("nc")
    if nc is None:
        nc = _build()
        _CACHE["nc"] = nc
    return nc


def _run(in_maps, trace=False, tmpdir=None):
    from concourse.bass_utils import run_bass_kernel_spmd

    nc = _get_nc()
    return run_bass_kernel_spmd(
        nc, in_maps, core_ids=list(range(NCORES)), trace=trace, tmpdir=tmpdir)


def make_in_maps(input, label, weight):
    inp = np.asarray(input, dtype=np.float32)
    lab = np.asarray(label).astype(np.int64)
    w = np.asarray(weight, dtype=np.float32)

    wpad = np.concatenate([w, np.ones((CPAD - C, D), np.float32)], axis=0)
    rnorm = 1.0 / np.maximum(np.linalg.norm(wpad, axis=1), 1e-12)
    wn = wpad * rnorm[:, None]

    xnorm = 1.0 / np.maximum(np.linalg.norm(inp, axis=1), 1e-12)
    xn = inp * xnorm[:, None]

    # a_lb from exact f32 normalized rows
    cos_lb = np.einsum("bd,bd->b", xn, wn[lab], dtype=np.float64)
    a_lb = np.where(
        cos_lb > THRESH,
        np.cos(np.arccos(np.clip(cos_lb, -1.0, 1.0)) + MARGIN),
        cos_lb - MM_,
    ).astype(np.float32)
    # ACT bias: -a/sqrt(2), per partition; row r = j*128 + p
    na = np.ascontiguousarray((-a_lb * RSQ2).reshape(2, 128).T)   # [128, 2]

    # xt[p, kp, ko, j2, b] = XS * xn[j2*128 + b, kp*256 + ko*128 + p]
    xt = np.ascontiguousarray(
        (xn * XS).astype(_f8e4).T.reshape(2, 2, 128, 2, 128)
        .transpose(2, 0, 1, 3, 4))

    # wt chunk-major DoubleRow layout: per chunk block [128, 2, 2, cols]
    # with wt[p, kp, ko, c] = WS * wn[c0 + c, kp*256 + ko*128 + p]
    wt_full = ((wn * WS).astype(_f8e4).T
               .reshape(2, 2, 128, CPAD).transpose(2, 0, 1, 3))
    in_maps = []
    for i in range(NCORES):
        sl = wt_full[:, :, :, i * CSH:(i + 1) * CSH]
        blocks = []
        c0 = 0
        for cols in CHUNKS:
            blocks.append(sl[:, :, :, c0:c0 + cols].reshape(128, 4 * cols))
            c0 += cols
        in_maps.append(
            {"xt": xt, "na": na,
             "wt": np.ascontiguousarray(np.concatenate(blocks, axis=1))})
    return in_maps, (lab, a_lb)


def assemble(results, aux):
    lab, a_lb = aux
    s = np.concatenate(
        [results[i]["out"] for i in range(NCORES)], axis=1
    )[:, :C]
    full = s.astype(np.float32) * np.float32(K2 / (WS * XS)) - np.float32(SCALE)
    full[np.arange(B), lab] = (SCALE * a_lb).astype(np.float32)
    return full


def kernel(input, label, weight):
    in_maps, aux = make_in_maps(input, label, weight)
    res = _run(in_maps)
    return assemble(res.results, aux)
